# revision 13
# baseline (speedup 1.0000x reference)
"""Octahedral SHT on 8 NeuronCores (Bass/Tile).

Strategy: shard the 192 latitude rings across 8 cores (24 rings each). Each
ring's ragged DFT (nlon in 20..400) is cut into K=128 chunks, zero-padded;
the ring assignment is engineered so every core gets exactly 51 chunks
-> one uniform SPMD program. The per-ring Legendre weights are replicated
per chunk (Pw2), which folds the intra-ring chunk reduction into phase 2.
Each core computes a partial [m, l, bev] coefficient tensor over its own
rings; the host sums the 8 partials and assembles the complex output.

Precision: fp32 matmuls on the PE are 4x slower and hit a codegen limit
(single wait slot on the split LDWEIGHTS), so each fp32 operand is split
hi/lo into two fp16 tensors (x = hi + lo, |lo| <= 2^-11 |x|). fp16*fp16
products are exact in the fp32 PSUM accumulator, so 3 accumulating MMs
(hi*hi + hi*lo + lo*hi) reproduce the fp32 product to ~2^-22.

Phase 1 (per chunk c): G[c] = xpad[c].T @ [E_re | E_im][c]   (PE, N=256)
Flatten: G' [c, bev*256 + r*128 + m]  (SBUF->SBUF DMA into partition row c)
Phase 2 (per m): out[l, (bev,r)] = Pw2[m].T @ G'[:, (bev,r) at m] (PE, N=256)
"""
import numpy as np

NLAT, LMAX, MMAX = 192, 128, 128
B, V = 2, 64
BF = B * V            # 128 fused batch (b*64+v)
NCORES = 8
CHUNK = 128
CHUNKS_PER_CORE = 51
RINGS_PER_CORE = 24
MAX_NLON = 400
NPTS = 40320


def _octa_nlon():
    half = NLAT // 2
    north = np.array([4 * (i + 1) + 16 for i in range(half)], dtype=np.int64)
    return np.concatenate([north, north[::-1]])


def _ring_assignment():
    nlon = _octa_nlon()
    v = np.ceil(nlon / CHUNK).astype(int)
    cores = [[] for _ in range(NCORES)]
    for cls in (1, 2, 3, 4):
        ids = np.where(v == cls)[0]
        ids = ids[np.argsort(-nlon[ids], kind="stable")]
        fwd = True
        for start in range(0, len(ids), NCORES):
            blk = ids[start:start + NCORES]
            order = range(NCORES) if fwd else range(NCORES - 1, -1, -1)
            for c, rid in zip(order, blk):
                cores[c].append(int(rid))
            fwd = not fwd
    return cores, nlon


def _split16(a):
    hi = a.astype(np.float16)
    lo = (a - hi.astype(np.float32)).astype(np.float16)
    return hi, lo


def _build_core_inputs(core_rings, nlon, offs, x, E_re, E_im, PwT):
    """x: [BF, npts] f32. Returns xe [51,128,768] f16 and pw2 [128,51,256] f16.

    xe columns: [x_hi 0:128 | E_hi 128:384 | x_lo 384:512 | E_lo 512:768]
    pw2 columns: [pw_hi 0:128 | pw_lo 128:256]
    """
    xpad = np.zeros((CHUNKS_PER_CORE, CHUNK, BF), np.float32)
    E2 = np.zeros((CHUNKS_PER_CORE, CHUNK, 2 * MMAX), np.float32)
    Pw2 = np.zeros((MMAX, CHUNKS_PER_CORE, LMAX), np.float32)
    c = 0
    for r in core_rings:
        nl = int(nlon[r])
        o = int(offs[r])
        for j0 in range(0, nl, CHUNK):
            jlen = min(CHUNK, nl - j0)
            xpad[c, :jlen, :] = x[:, o + j0:o + j0 + jlen].T
            elen = min(CHUNK, MAX_NLON - j0)
            if elen > 0:
                E2[c, :elen, 0:MMAX] = E_re[r, j0:j0 + elen, :]
                E2[c, :elen, MMAX:] = E_im[r, j0:j0 + elen, :]
            Pw2[:, c, :] = PwT[:, r, :]
            c += 1
    assert c == CHUNKS_PER_CORE
    xh, xl = _split16(xpad)
    eh, el = _split16(E2)
    xe = np.concatenate([xh, eh, xl, el], axis=2)  # [51, 128, 768]
    ph, pl = _split16(Pw2)
    pw2 = np.concatenate([ph, pl], axis=2)         # [128, 51, 256]
    return np.ascontiguousarray(xe), np.ascontiguousarray(pw2)


def _build_bass():
    import concourse.bass as bass
    import concourse.mybir as mybir
    from concourse import bacc, tile

    dt = mybir.dt
    nc = bacc.Bacc()

    xe_d = nc.dram_tensor("xe", [CHUNKS_PER_CORE, CHUNK, 768], dt.float16,
                          kind="ExternalInput")
    pw2_d = nc.dram_tensor("pw2", [MMAX, CHUNKS_PER_CORE, 2 * LMAX], dt.float16,
                           kind="ExternalInput")
    outp_d = nc.dram_tensor("outp", [MMAX, LMAX, 2 * BF], dt.float32,
                            kind="ExternalOutput")

    with tile.TileContext(nc) as tc:
        with (
            tc.tile_pool(name="xs", bufs=8) as xs_pool,
            tc.tile_pool(name="gt", bufs=4) as gt_pool,
            tc.tile_pool(name="gs", bufs=1) as gs_pool,
            tc.tile_pool(name="pws", bufs=8) as pw_pool,
            tc.tile_pool(name="os", bufs=4) as os_pool,
            tc.tile_pool(name="ps1", bufs=4, space="PSUM") as ps1,
            tc.tile_pool(name="ps2", bufs=4, space="PSUM") as ps2,
        ):
            gsbh = gs_pool.tile([CHUNKS_PER_CORE, BF * 2 * MMAX], dt.float16)
            gsbl = gs_pool.tile([CHUNKS_PER_CORE, BF * 2 * MMAX], dt.float16)

            # ---- phase 1: 51 chunks x 3 accumulating MMs ----
            for c in range(CHUNKS_PER_CORE):
                xe = xs_pool.tile([CHUNK, 768], dt.float16, tag="xe")
                nc.sync.dma_start(out=xe[:], in_=xe_d[c])
                xh = xe[:, 0:128]
                eh = xe[:, 128:384]
                xl = xe[:, 384:512]
                el = xe[:, 512:768]
                g_ps = ps1.tile([BF, 2 * MMAX], dt.float32, tag="g")
                nc.tensor.matmul(g_ps[:], xh, eh, start=True, stop=False)
                nc.tensor.matmul(g_ps[:], xh, el, start=False, stop=False)
                nc.tensor.matmul(g_ps[:], xl, eh, start=False, stop=True)
                # evacuate PSUM, splitting fp32 -> fp16 hi + lo (both on DVE
                # so psum slot recycling costs the MM a single extra sem)
                g_hi = gt_pool.tile([BF, 2 * MMAX], dt.float16, tag="ghi")
                g_lo = gt_pool.tile([BF, 2 * MMAX], dt.float16, tag="glo")
                nc.vector.tensor_copy(g_hi[:], g_ps[:])
                nc.vector.tensor_sub(g_lo[:], g_ps[:], g_hi[:])
                # flatten: G'[c, bev*256 + rm] <- [bev, rm] (partition-major)
                nc.sync.dma_start(out=gsbh[c:c + 1, :], in_=g_hi[:])
                nc.sync.dma_start(out=gsbl[c:c + 1, :], in_=g_lo[:])

            # all-engine barrier: phase-2 MMs read G' written by 8 DMA queues;
            # without this each MM would need too many waits.
            tc.strict_bb_all_engine_barrier()

            # ---- phase 2: 128 m x 3 accumulating MMs ----
            gvh = gsbh[:].rearrange("c (bev r m) -> c bev r m", bev=BF, r=2, m=MMAX)
            gvl = gsbl[:].rearrange("c (bev r m) -> c bev r m", bev=BF, r=2, m=MMAX)
            for m in range(MMAX):
                pwt = pw_pool.tile([CHUNKS_PER_CORE, 2 * LMAX], dt.float16, tag="pw")
                nc.sync.dma_start(out=pwt[:], in_=pw2_d[m])
                ph = pwt[:, 0:128]
                pl = pwt[:, 128:256]
                rhs_h = gvh[:, :, :, m]  # [c=51, bev=128, r=2] free pattern
                rhs_l = gvl[:, :, :, m]
                o_ps = ps2.tile([LMAX, 2 * BF], dt.float32, tag="o")
                nc.tensor.matmul(o_ps[:], ph, rhs_h, start=True, stop=False)
                nc.tensor.matmul(o_ps[:], ph, rhs_l, start=False, stop=False)
                nc.tensor.matmul(o_ps[:], pl, rhs_h, start=False, stop=True)
                o_sb = os_pool.tile([LMAX, 2 * BF], dt.float32, tag="ot")
                if m % 2 == 0:
                    nc.vector.tensor_copy(o_sb[:], o_ps[:])
                else:
                    nc.scalar.copy(o_sb[:], o_ps[:])
                nc.sync.dma_start(out=outp_d[m], in_=o_sb[:])

    nc.compile()
    return nc


_CACHE = {}


def _get_compiled():
    if "nc" not in _CACHE:
        _CACHE["nc"] = _build_bass()
    return _CACHE["nc"]


def kernel(data, Pw, E_re, E_im, pad_idx):
    from concourse import bass_utils

    data = np.asarray(data)
    Pw = np.asarray(Pw, dtype=np.float32)
    E_re = np.asarray(E_re, dtype=np.float32)
    E_im = np.asarray(E_im, dtype=np.float32)

    cores, nlon = _ring_assignment()
    offs = np.concatenate([[0], np.cumsum(nlon)[:-1]])
    # 'b e p v -> (b e v) p'
    x = np.ascontiguousarray(
        np.transpose(data, (0, 1, 3, 2)).reshape(BF, NPTS).astype(np.float32))
    PwT = np.ascontiguousarray(np.transpose(Pw, (1, 2, 0)))  # [m, n, l]

    in_maps = []
    for c in range(NCORES):
        xe, pw2 = _build_core_inputs(cores[c], nlon, offs, x, E_re, E_im, PwT)
        in_maps.append({"xe": xe, "pw2": pw2})

    nc = _get_compiled()
    res = bass_utils.run_bass_kernel_spmd(nc, in_maps, list(range(NCORES)))
    _CACHE["last_results"] = res

    total = np.zeros((MMAX, LMAX, 2 * BF), np.float64)
    for r in res.results:
        total += r["outp"].astype(np.float64)
    total = total.astype(np.float32).reshape(MMAX, LMAX, BF, 2)
    cc = total[..., 0] + 1j * total[..., 1]        # [m, l, bev]
    cc = cc.reshape(MMAX, LMAX, B, V)
    out = np.transpose(cc, (2, 1, 0, 3))[:, None]  # [b, 1, l, m, v]
    return out.astype(np.complex64)


# revision 14
# speedup vs baseline: 1.2588x; 1.2588x over previous
"""Octahedral SHT on 8 NeuronCores (Bass/Tile).

Strategy: shard the 192 latitude rings across 8 cores (24 rings each). Each
ring's ragged DFT (nlon in 20..400) is cut into K=128 chunks, zero-padded;
the ring assignment is engineered so every core gets exactly 51 chunks
-> one uniform SPMD program. The per-ring Legendre weights are replicated
per chunk, which folds the intra-ring chunk reduction into phase 2.
Each core computes a partial [l, m, bev] coefficient tensor over its own
rings; the host sums the 8 partials and assembles the complex output.

Precision: fp32 matmuls on the PE are 4x slower, so each fp32 operand is
split hi/lo into two fp16 tensors (x = hi + lo, |lo| <= 2^-11 |x|).
fp16*fp16 products are exact in the fp32 PSUM accumulator, so accumulating
MMs (hi*hi + hi*lo + lo*hi) reproduce the fp32 product to ~2^-22.

Phase 1 (per chunk c): G[c] = xpad[c].T @ [E_re | E_im][c]   (PE, N=256, 3 MMs)
Flatten: G'hi[row c] / G'lo[row 51+c] <- [bev, rm] (SBUF->SBUF DMA, one
  partition row each; row layout bev*256 + r*128 + m)
Phase 2 (per m): out[l, (bev,r)] = 2 MMs:
  MM_a: K=102 lhsT=[pw_hi;pw_hi] rhs=G'[0:102]  (hi*hi + hi*lo fused)
  MM_b: K=51  lhsT=pw_lo         rhs=G'[0:51]   (lo*hi)
"""
import numpy as np

NLAT, LMAX, MMAX = 192, 128, 128
B, V = 2, 64
BF = B * V            # 128 fused batch (b*64+v)
NCORES = 8
CHUNK = 128
NCH = 51              # chunks per core
RINGS_PER_CORE = 24
MAX_NLON = 400
NPTS = 40320
GB = [0, 13, 26, 39, NCH]   # xe load group bounds
MG = 4                      # m's per psum/out group
PWG = 16                    # m's per pw load group


def _octa_nlon():
    half = NLAT // 2
    north = np.array([4 * (i + 1) + 16 for i in range(half)], dtype=np.int64)
    return np.concatenate([north, north[::-1]])


def _ring_assignment():
    nlon = _octa_nlon()
    v = np.ceil(nlon / CHUNK).astype(int)
    cores = [[] for _ in range(NCORES)]
    for cls in (1, 2, 3, 4):
        ids = np.where(v == cls)[0]
        ids = ids[np.argsort(-nlon[ids], kind="stable")]
        fwd = True
        for start in range(0, len(ids), NCORES):
            blk = ids[start:start + NCORES]
            order = range(NCORES) if fwd else range(NCORES - 1, -1, -1)
            for c, rid in zip(order, blk):
                cores[c].append(int(rid))
            fwd = not fwd
    return cores, nlon


def _split16(a):
    hi = a.astype(np.float16)
    lo = (a - hi.astype(np.float32)).astype(np.float16)
    return hi, lo


def _build_core_inputs(core_rings, nlon, offs, x, E_re, E_im, PwT):
    """x: [BF, npts] f32.  Returns:
    xe  [128 j, 51 c, 768] f16  cols: [x_hi 128 | E_hi 256 | x_lo 128 | E_lo 256]
    pw  [102, 128 m, 256] f16   rows 0-50: [pw_hi | pw_lo], rows 51-101: [pw_hi | 0]
    """
    xpad = np.zeros((NCH, CHUNK, BF), np.float32)
    E2 = np.zeros((NCH, CHUNK, 2 * MMAX), np.float32)
    Pw2 = np.zeros((MMAX, NCH, LMAX), np.float32)
    c = 0
    for r in core_rings:
        nl = int(nlon[r])
        o = int(offs[r])
        for j0 in range(0, nl, CHUNK):
            jlen = min(CHUNK, nl - j0)
            xpad[c, :jlen, :] = x[:, o + j0:o + j0 + jlen].T
            elen = min(CHUNK, MAX_NLON - j0)
            if elen > 0:
                E2[c, :elen, 0:MMAX] = E_re[r, j0:j0 + elen, :]
                E2[c, :elen, MMAX:] = E_im[r, j0:j0 + elen, :]
            Pw2[:, c, :] = PwT[:, r, :]
            c += 1
    assert c == NCH
    xh, xl = _split16(xpad)
    eh, el = _split16(E2)
    xe = np.concatenate([xh, eh, xl, el], axis=2)   # [51, 128, 768]
    xe = np.ascontiguousarray(xe.transpose(1, 0, 2))  # [128 j, 51 c, 768]

    ph, pl = _split16(Pw2)                           # [m, c, l] each
    pw = np.zeros((102, MMAX, 2 * LMAX), np.float16)
    pw[0:51, :, 0:128] = ph.transpose(1, 0, 2)       # pw_hi
    pw[0:51, :, 128:256] = pl.transpose(1, 0, 2)     # pw_lo
    pw[51:102, :, 0:128] = ph.transpose(1, 0, 2)     # pw_hi again (K-pack)
    return xe, pw


def _build_bass():
    import concourse.bass as bass
    import concourse.mybir as mybir
    from concourse import bacc, tile

    dt = mybir.dt
    nc = bacc.Bacc()

    xe_d = nc.dram_tensor("xe", [CHUNK, NCH, 768], dt.float16,
                          kind="ExternalInput")
    pw_d = nc.dram_tensor("pw", [102, MMAX, 2 * LMAX], dt.float16,
                          kind="ExternalInput")
    outp_d = nc.dram_tensor("outp", [LMAX, MMAX, 2 * BF], dt.float32,
                            kind="ExternalOutput")

    with tile.TileContext(nc) as tc:
        with (
            tc.tile_pool(name="xs", bufs=2) as xs_pool,
            tc.tile_pool(name="gt", bufs=4) as gt_pool,
            tc.tile_pool(name="gs", bufs=1) as gs_pool,
            tc.tile_pool(name="pws", bufs=2) as pw_pool,
            tc.tile_pool(name="os", bufs=3) as os_pool,
            tc.tile_pool(name="ps1", bufs=3, space="PSUM") as ps1,
            tc.tile_pool(name="ps2", bufs=2, space="PSUM") as ps2,
        ):
            # G' hi rows 0-50, lo rows 51-101; row layout bev*256 + r*128 + m
            gsb = gs_pool.tile([102, BF * 2 * MMAX], dt.float16)

            # ---- phase 1: 51 chunks x 3 accumulating MMs ----
            xg = {}
            for g in range(4):
                n = GB[g + 1] - GB[g]
                t = xs_pool.tile([CHUNK, n * 768], dt.float16, tag="xg")
                nc.sync.dma_start(out=t[:], in_=xe_d[:, GB[g]:GB[g + 1], :])
                xg[g] = t

            for c in range(NCH):
                g = next(i for i in range(4) if GB[i] <= c < GB[i + 1])
                off = (c - GB[g]) * 768
                xe = xg[g]
                xh = xe[:, off + 0:off + 128]
                eh = xe[:, off + 128:off + 384]
                xl = xe[:, off + 384:off + 512]
                el = xe[:, off + 512:off + 768]
                g_ps = ps1.tile([BF, 2 * MMAX], dt.float32, tag="g")
                nc.tensor.matmul(g_ps[:], xh, eh, start=True, stop=False)
                nc.tensor.matmul(g_ps[:], xh, el, start=False, stop=False)
                nc.tensor.matmul(g_ps[:], xl, eh, start=False, stop=True)
                # evacuate PSUM, splitting fp32 -> fp16 hi (ACT) + lo (DVE)
                g_hl = gt_pool.tile([BF, 512], dt.float16, tag="ghl")
                nc.scalar.copy(g_hl[:, 0:256], g_ps[:])
                nc.vector.tensor_sub(g_hl[:, 256:512], g_ps[:], g_hl[:, 0:256])
                # flatten into partition rows (SWDGE: keeps SP free)
                nc.gpsimd.dma_start(out=gsb[c:c + 1, :], in_=g_hl[:, 0:256])
                nc.gpsimd.dma_start(out=gsb[51 + c:52 + c, :], in_=g_hl[:, 256:512])

            # ---- phase 2: 128 m x 2 accumulating MMs ----
            gva = gsb[0:102].rearrange("c (bev r m) -> c bev r m", bev=BF, r=2, m=MMAX)
            gvb = gsb[0:51].rearrange("c (bev r m) -> c bev r m", bev=BF, r=2, m=MMAX)
            for mg in range(0, MMAX, PWG):
                pwt = pw_pool.tile([102, PWG * 256], dt.float16, tag="pw")
                nc.sync.dma_start(out=pwt[:], in_=pw_d[:, mg:mg + PWG, :])
                for m4 in range(mg, mg + PWG, MG):
                    o_ps = ps2.tile([LMAX, MG * 256], dt.float32, tag="o")
                    for m in range(m4, m4 + MG):
                        mo = (m - mg) * 256
                        po = (m - m4) * 256
                        pa = pwt[:, mo + 0:mo + 128]
                        pb = pwt[0:51, mo + 128:mo + 256]
                        nc.tensor.matmul(o_ps[:, po:po + 256], pa,
                                         gva[:, :, :, m], start=True, stop=False)
                        nc.tensor.matmul(o_ps[:, po:po + 256], pb,
                                         gvb[:, :, :, m], start=False, stop=True)
                    o_sb = os_pool.tile([LMAX, MG * 256], dt.float32, tag="ot")
                    if (m4 // MG) % 2 == 0:
                        nc.vector.tensor_copy(o_sb[:], o_ps[:])
                    else:
                        nc.scalar.copy(o_sb[:], o_ps[:])
                    nc.sync.dma_start(out=outp_d[:, m4:m4 + MG, :], in_=o_sb[:])

    nc.compile()
    return nc


_CACHE = {}


def _get_compiled():
    if "nc" not in _CACHE:
        _CACHE["nc"] = _build_bass()
    return _CACHE["nc"]


def kernel(data, Pw, E_re, E_im, pad_idx):
    from concourse import bass_utils

    data = np.asarray(data)
    Pw = np.asarray(Pw, dtype=np.float32)
    E_re = np.asarray(E_re, dtype=np.float32)
    E_im = np.asarray(E_im, dtype=np.float32)

    cores, nlon = _ring_assignment()
    offs = np.concatenate([[0], np.cumsum(nlon)[:-1]])
    # 'b e p v -> (b e v) p'
    x = np.ascontiguousarray(
        np.transpose(data, (0, 1, 3, 2)).reshape(BF, NPTS).astype(np.float32))
    PwT = np.ascontiguousarray(np.transpose(Pw, (1, 2, 0)))  # [m, n, l]

    in_maps = []
    for c in range(NCORES):
        xe, pw = _build_core_inputs(cores[c], nlon, offs, x, E_re, E_im, PwT)
        in_maps.append({"xe": xe, "pw": pw})

    nc = _get_compiled()
    res = bass_utils.run_bass_kernel_spmd(nc, in_maps, list(range(NCORES)))
    _CACHE["last_results"] = res

    total = np.zeros((LMAX, MMAX, 2 * BF), np.float64)
    for r in res.results:
        total += r["outp"].astype(np.float64)
    total = total.astype(np.float32).reshape(LMAX, MMAX, BF, 2)
    cc = total[..., 0] + 1j * total[..., 1]        # [l, m, bev]
    cc = cc.reshape(LMAX, MMAX, B, V)
    out = np.transpose(cc, (2, 0, 1, 3))[:, None]  # [b, 1, l, m, v]
    return out.astype(np.complex64)
